# revision 73
# baseline (speedup 1.0000x reference)
"""Trainium2 Bass kernel for nn_MultiHeadAttention_9878424780806.

Problem (hardcoded): B=4, S=2048, D=1024, H=16 heads (head_dim 64), fp32.
  y = softmax((x@Wq)(x@Wk)^T / 8 + mask) @ (x@Wv) @ Wout   (+ zero biases)

Sharding: 8 cores = 4 batches x 2 head-halves (8 heads per core). Each core
computes a partial y for its batch from its 8 heads; the host sums the two
partials per batch (the out-projection is linear in heads). Host feeds x
pre-transposed per batch (xT [D, S]) and converts everything to bf16
(tolerance 2e-2 >> bf16 matmul error; halves DMA traffic and SBUF).

Per-core structure: 16 attention windows (8 heads x 2 q-halves of 1024),
fused into one 256-step software-pipelined stream. At step s:
  scores(s+1) [PE, bf16 -> fp32 PSUM] -> exp(s) [ScalarE, scale=1/8, bf16
  out] -> filler chunks -> attn@[V|1](s-lag): the probs tile is the
  STATIONARY operand, so out pu[q, 65] puts q on partitions and the softmax
  row-sum (from the appended ones column) at free col 64 -- normalization
  is then a per-partition scalar multiply (DVE reciprocal + broadcast mul),
  no cross-partition traffic.
The exp stream is the critical path (1038ns/step: 1 elem/lane/cycle at
1.2GHz + fixed access overhead; PSUM bank budget forbids wider tiles) and
runs gap-free: startup is DMA-latency-trimmed (pair-major host layouts
give 2KB descriptors; the serial DMA engine carries only what the first
projections need, split so each matmul's d-tiles can start as they land),
and V-projections are emitted per (pair, k-tile) so each pair's 16 units
are consumed by its own windows -- projecting all heads up front
concentrated a ~27us PE debt in window 0 and starved ScalarE. attn@V
emission lags the exp stream (ramp to ~20 steps, repaid through the
middle) to push that debt into per-step slack.

Normalized ao (pair-packed [128q, qt, 128f], both heads) is transposed by
PE into aoT [128f, t]. Out-projection runs in pair-groups writing three
outputs (y = pairs 0+1, y2 = pair 2, y3 = pair 3; host sums them): y/y2
stream out as fillers; ALL of y3 runs post-stream as the tail, when
ScalarE/DVE are free to alternate full-width PSUM->SBUF copies (GPSIMD
cannot touch PSUM on hw). Filler generators yield every ~2 matmuls and are
pumped 2 chunks/step in FIFO order arranged to avoid head-of-line blocking
on late x-chunk DMAs; scores matmuls are emitted at high priority so they
always beat the backlog. PSUM: scores 2x2 banks, pu 2x1, two 1-bank
scratch slots; the tail rotates over scores+scratch (3 slots).

All matmuls are bf16 (1 col/cycle at any width; rel-err ~5e-3 vs the 2e-2
gate); attn_mask and biases are all-zero by construction (spec fill=zeros);
kernel() refuses non-zero values.

Cost-model timeline: 310840ns (session start) -> 296444ns: exp stream is
gap-free from first exp at 12.8us to ~280us; y3 tail (ScalarE-leaning
full-width copies + serial 11.6us DMA drain) + close-out ~16.5us.
"""

import numpy as np
from collections import deque
from contextlib import ExitStack

import concourse.bass as bass
import concourse.tile as tile
from concourse import bacc, mybir
from concourse.bass_utils import run_bass_kernel_spmd
from concourse.masks import make_identity

F32 = mybir.dt.float32
BF16 = mybir.dt.bfloat16

B, S, D, H = 4, 2048, 1024, 16
HD = 64          # head dim
HPC = 8          # heads per core
N_CORES = 8

TT = S // 128    # 16 token tiles
DT = D // 128    # 8 d tiles
FH = HPC * HD    # 512 features per core half
NPAIR = HPC // 2
QH = S // 2      # 1024-wide q half
QT = QH // 128   # 8 q tiles per half
LAG_MAX = 20     # max attn@V emission lag (steps); bounded by atp bufs

# Host-side layouts are chosen so every load's innermost contiguous run is
# >= 2KB (descriptor elem >= 512B avoids the 2x DMA latency multiplier):
#   xt   [4, 128, DT, 512]  chunk-major x^T   (c,p,dt,t) = x.T[dt*128+p, c*512+t]
#   wq/wk[NPAIR, 128, DT, 128] pair-major     (j,p,dt,f) = W[dt*128+p, j*128+f]
#   wv   [128, DT, FH]                        (p,dt,f)   = Wv[dt*128+p, f]
#   wout [FH, D]
IN_SPECS = [("xt", [4, 128, DT, 512], BF16),
            ("wq", [NPAIR, 128, DT, 128], BF16),
            ("wk", [NPAIR, 128, DT, 128], BF16),
            ("wv", [NPAIR, 128, DT, 128], BF16),
            ("wout", [FH, D], BF16)]
OUT_SPECS = [("y", [S, D], BF16), ("y2", [S, D], BF16),
             ("y3", [S, D], BF16)]


def build_program():
    nc = bacc.Bacc("TRN2", target_bir_lowering=False, debug=False,
                   enable_asserts=False, num_devices=N_CORES)

    xt_ap = nc.dram_tensor("xt", [4, 128, DT, 512], BF16,
                           kind="ExternalInput").ap()
    wq_ap = nc.dram_tensor("wq", [NPAIR, 128, DT, 128], BF16,
                           kind="ExternalInput").ap()
    wk_ap = nc.dram_tensor("wk", [NPAIR, 128, DT, 128], BF16,
                           kind="ExternalInput").ap()
    wv_ap = nc.dram_tensor("wv", [NPAIR, 128, DT, 128], BF16,
                           kind="ExternalInput").ap()
    wout_ap = nc.dram_tensor("wout", [FH, D], BF16, kind="ExternalInput").ap()
    y_ap = nc.dram_tensor("y", [S, D], BF16, kind="ExternalOutput").ap()
    y2_ap = nc.dram_tensor("y2", [S, D], BF16, kind="ExternalOutput").ap()
    y3_ap = nc.dram_tensor("y3", [S, D], BF16, kind="ExternalOutput").ap()

    wout_r = wout_ap.rearrange("(ft p) e -> p ft e", p=128)

    with tile.TileContext(nc) as tc, ExitStack() as ctx:
        # SBUF pools
        xtp = ctx.enter_context(tc.tile_pool(name="xTp", bufs=1))
        wqkp = ctx.enter_context(tc.tile_pool(name="wqkp", bufs=2))
        wvp = ctx.enter_context(tc.tile_pool(name="wvp", bufs=1))
        wop = ctx.enter_context(tc.tile_pool(name="wop", bufs=1))
        qkp = ctx.enter_context(tc.tile_pool(name="qkp", bufs=2))
        v1p = ctx.enter_context(tc.tile_pool(name="v1p", bufs=1))
        atp = ctx.enter_context(tc.tile_pool(name="atp", bufs=34))
        aop = ctx.enter_context(tc.tile_pool(name="aop", bufs=6))
        aotp = ctx.enter_context(tc.tile_pool(name="aotp", bufs=3))
        rp = ctx.enter_context(tc.tile_pool(name="rp", bufs=4))
        ysbp = ctx.enter_context(tc.tile_pool(name="ysbp", bufs=8))
        idp = ctx.enter_context(tc.tile_pool(name="idp", bufs=1))
        # PSUM pools: 4 + 2 + 1 + 1 = 8 banks exactly
        psp = ctx.enter_context(tc.tile_pool(name="psp", bufs=2, space="PSUM"))
        pup = ctx.enter_context(tc.tile_pool(name="pup", bufs=1, space="PSUM"))
        psyp = ctx.enter_context(tc.tile_pool(name="psyp", bufs=1, space="PSUM"))
        psqp = ctx.enter_context(tc.tile_pool(name="psqp", bufs=1, space="PSUM"))

        # ramp the PE p-state while the input DMAs land: enough dummy
        # matmuls to still be running when the first projection starts,
        # so the ramp (3us of continuous busy) carries into real work.
        # A memset dummy (not the identity) so PE isn't gated on the
        # DVE-built identity tile.
        dummy = idp.tile([1, 128], BF16, tag="dummy")
        nc.vector.memset(dummy[:, :], 0.0)
        pwarm = psyp.tile([128, 128], F32, tag="psy", name="pwarm")
        for _ in range(46):
            nc.tensor.matmul(pwarm[:, :], dummy[0:1, :], dummy[0:1, :],
                             start=True, stop=True)

        # ---- loads ----
        # Emission order = DMA-engine order; the serial DMA engine is the
        # startup critical path: k0 and xt0 gate the first k-projection,
        # wq0 the q-projections, xt1 the second q chunk.
        wqk_t = [None] * NPAIR

        def emit_wqk_load(j):
            # [128, {q,k}, dt, 128]: each half is contiguous per partition
            # (2KB descriptors); k half first (k-proj is the first consumer)
            wqk = wqkp.tile([128, 2, DT, 128], BF16, tag="wqk",
                            name=f"wqk{j}")
            nc.sync.dma_start(wqk[:, 1], wk_ap[j])
            nc.sync.dma_start(wqk[:, 0], wq_ap[j])
            wqk_t[j] = wqk

        xTc = [xtp.tile([128, DT, 512], BF16, tag=f"xt{c}", name=f"xt{c}")
               for c in range(4)]
        wqk0 = wqkp.tile([128, 2, DT, 128], BF16, tag="wqk", name="wqk0")
        wqk_t[0] = wqk0
        nc.sync.dma_start(wqk0[:, 1], wk_ap[0])
        nc.sync.dma_start(xTc[0][:, 0:4, :], xt_ap[0, :, 0:4, :])
        nc.sync.dma_start(xTc[0][:, 4:8, :], xt_ap[0, :, 4:8, :])
        nc.sync.dma_start(wqk0[:, 0], wq_ap[0])
        nc.sync.dma_start(xTc[1][:, 0:4, :], xt_ap[1, :, 0:4, :])
        nc.sync.dma_start(xTc[1][:, 4:8, :], xt_ap[1, :, 4:8, :])

        # wv split per pair: pair 0's slice rides ahead of the cold x
        # chunks (its V-projections start first); the rest follows xt2/xt3
        # so the k-projections for late k-tiles aren't starved
        wv_t = wvp.tile([128, NPAIR, DT, 128], BF16, tag="wv")
        nc.sync.dma_start(wv_t[:, 0], wv_ap[0])

        for c in range(2, 4):
            nc.sync.dma_start(xTc[c][:, :, :], xt_ap[c])

        for j in range(1, NPAIR):
            nc.sync.dma_start(wv_t[:, j], wv_ap[j])

        wout_t = []
        for j in range(NPAIR):
            w = wop.tile([128, 1024], BF16, tag=f"wout{j}")
            nc.sync.dma_start(w[:, :], wout_r[:, j, :])
            wout_t.append(w)

        idt = idp.tile([128, 128], BF16, tag="id")
        make_identity(nc, idt[:, :])

        # preload the Exp activation table while DMAs are in flight
        warm = rp.tile([1, 2], F32, tag="warm")
        nc.vector.memset(warm[:, :], 0.0)
        nc.scalar.activation(warm[:, 1:2], warm[:, 0:1],
                             func=mybir.ActivationFunctionType.Exp)

        # ---- filler machinery ----
        # Fillers are generators that yield every ~2 matmuls (~430ns of PE
        # work). The attention windows pump one chunk per k-tile, matching
        # the PE slack left over while ScalarE exps; hard dependencies
        # (projections an upcoming scores matmul needs) are force-completed.
        fillers = deque()
        _cur = [None]

        def pump(chunks=1):
            while chunks > 0:
                if _cur[0] is None:
                    if not fillers:
                        return
                    _cur[0] = fillers.popleft()
                try:
                    next(_cur[0])
                    chunks -= 1
                except StopIteration:
                    _cur[0] = None

        def force(gen):
            for _ in gen:
                pass

        def drain():
            pump(1 << 30)

        # QK projection: qkT[j] [128f, {q,k}, S]
        qkT = [None] * NPAIR
        qk_gen = {}

        def qkproj_gen(j, fs, tck, sel):
            pool, tg = (psqp, "psq") if sel % 2 == 0 else (psyp, "psy")
            psq = pool.tile([128, 512], F32, tag=tg, name="psq")
            for dt in range(DT):
                nc.tensor.matmul(
                    psq[:, :],
                    wqk_t[j][:, fs, dt, :],
                    xTc[tck][:, dt, :],
                    start=(dt == 0), stop=(dt == DT - 1))
                if dt % 2 == 1 and dt < DT - 1:
                    yield
            nc.vector.tensor_copy(
                qkT[j][:, fs, tck * 512:(tck + 1) * 512], psq[:, :])
            yield

        def queue_qkproj(j, order=None):
            if j > 0:
                emit_wqk_load(j)
            qkT[j] = qkp.tile([128, 2, S], BF16, tag="qkT", name=f"qkT{j}")
            # k (fs=1) before q, early token chunks first: matches the order
            # the first window consumes them in
            for i, (fs, tck) in enumerate(order or
                                          [(1, 0), (0, 0), (0, 1), (1, 1),
                                           (1, 2), (1, 3), (0, 2), (0, 3)]):
                g = qkproj_gen(j, fs, tck, i)
                qk_gen[(j, fs, tck)] = g
                fillers.append(g)

        # V projection per (pair, k-tile): v1k[(j,kt)] [128t, 2h, 65] =
        # [V | 1]. Split per pair so pair j's 16 units are only consumed by
        # its own 4 windows -- projecting all 8 heads up front concentrated
        # a ~27us PE debt into window 0 and stalled the exp stream early.
        v1k = {}
        v_gen = {}

        def vproj_gen(j, kt):
            v1 = v1p.tile([128, 2, HD + 1], BF16, tag=f"v1_{j}_{kt}",
                          name=f"v1_{j}_{kt}")
            v1k[(j, kt)] = v1
            nc.vector.memset(v1[:, :, HD], 1.0)
            xc, sub = xTc[kt // 4], kt % 4
            pool, tg = (psqp, "psq") if kt % 2 == 0 else (psyp, "psy")
            psv = pool.tile([128, 128], F32, tag=tg, name="psv")
            for dt in range(DT):
                nc.tensor.matmul(psv[:, :],
                                 xc[:, dt, sub * 128:(sub + 1) * 128],
                                 wv_t[:, j, dt, :],
                                 start=(dt == 0), stop=(dt == DT - 1))
                if dt % 2 == 1 and dt < DT - 1:
                    yield
            nc.vector.tensor_copy(
                v1[:, :, 0:HD],
                psv[:, :].rearrange("p (h c) -> p h c", c=HD))
            yield

        def queue_vproj(j):
            for kt in range(TT):
                g = vproj_gen(j, kt)
                v_gen[(j, kt)] = g
                fillers.append(g)

        # Transposes: pair-packed ao [128q, 8qt, 128f] -> aoT[pair] [128f, S]
        # via PE transpose ([128,128] per qt) into scratch PSUM, then one DVE
        # copy per 4-qt half.
        aoT = [None] * NPAIR

        def transpose_gen(j, qh, ao, half):
            pool, tg = (psqp, "psq") if half == 0 else (psyp, "psy")
            trT = pool.tile([128, 4, 128], BF16, tag=tg, name="trT")
            for qi in range(4):
                qt = half * 4 + qi
                nc.tensor.transpose(trT[:, qi, :], ao[:, qt, :], idt[:, :])
                if qi == 1:
                    yield
            nc.vector.tensor_copy(
                aoT[j].rearrange("p (t c) -> p t c", c=128)
                    [:, qh * 8 + half * 4:qh * 8 + half * 4 + 4, :],
                trT[:, :, :])
            yield

        def transpose_pe(j, qh, ao):
            # synchronous variant for the tail unit
            for half in range(2):
                for _ in transpose_gen(j, qh, ao, half):
                    pass

        # Out-projection in pair-groups so most units can run as filler while
        # attention is still in flight: pairs {0,1} write y, then {2} and {3}
        # DMA-accumulate on top as their aoT becomes available.
        def outproj_gen(pairs, tt, ec, target):
            pool, tg = (psyp, "psy") if (tt + ec) % 2 == 0 else (psqp, "psq")
            psy = pool.tile([128, 512], F32, tag=tg, name="psy")
            for i, j in enumerate(pairs):
                nc.tensor.matmul(
                    psy[:, :],
                    aoT[j].rearrange("p (t c) -> p t c", c=128)[:, tt, :],
                    wout_t[j][:, ec * 512:(ec + 1) * 512],
                    start=(i == 0), stop=(i == len(pairs) - 1))
            ysb = ysbp.tile([128, 512], BF16, tag="y", name="ysb")
            nc.vector.tensor_copy(ysb[:, :], psy[:, :])
            nc.sync.dma_start(
                target[tt * 128:(tt + 1) * 128, ec * 512:(ec + 1) * 512],
                ysb[:, :])
            yield

        # Tail variant (pair 3, q-half 1): attention is done, so
        # double-buffer across the (now free) scores pool. Copies are split
        # per 512-col half and spread over Pool+DVE+ACT so the copy latency
        # (the tail's pacing term) is ~650ns instead of 1.2us, and each half
        # DMAs out as soon as its copy lands.
        def tail_copy(eng, dst, src):
            # GPSIMD cannot read PSUM on hw: ACT/DVE only
            [nc.scalar.copy, nc.vector.tensor_copy][eng % 2](dst, src)

        def emit_outproj_tail(tt, i):
            aoTv = aoT[3].rearrange("p (t c) -> p t c", c=128)
            if i % 3 == 2:
                halves = [psqp.tile([128, 512], F32, tag="psq", name="pst"),
                          psyp.tile([128, 512], F32, tag="psy", name="pst")]
                wide = None
            else:
                psy = psp.tile([128, 1024], F32, tag="ps", name="psy2")
                halves = [psy[:, 0:512], psy[:, 512:1024]]
                wide = psy
            ysb = ysbp.tile([128, 1024], BF16, tag="ytail", name="ysb2")
            for ec in range(2):
                nc.tensor.matmul(
                    halves[ec], aoTv[:, tt, :],
                    wout_t[3][:, ec * 512:(ec + 1) * 512],
                    start=True, stop=True)
            if wide is not None:
                # one full-width copy amortizes the per-instruction access
                # overhead. The first three tiles go to ScalarE: it frees the
                # moment the last exp retires, while DVE still carries the
                # final normalize + transpose copies; later tiles lean 2-of-3
                # onto ScalarE (its copy is 150ns cheaper than DVE's).
                tail_copy(0 if i < 3 or i % 3 != 1 else 1, ysb[:, :], wide[:, :])
            else:
                tail_copy(i, ysb[:, 0:512], halves[0])
                tail_copy(i + 1, ysb[:, 512:1024], halves[1])
            nc.sync.dma_start(y3_ap[tt * 128:(tt + 1) * 128, :], ysb[:, :])

        # ---- attention stream: 16 windows (pair-major, both-heads-q0
        # first) fused into one 256-step software pipeline. At step s:
        # scores(s+1) -> exp(s) -> filler chunk -> attn@V(s-1); window
        # boundaries are pipelined over just like k-tile boundaries.
        WINS = [(j, hs, qh) for j in range(NPAIR)
                for hs, qh in [(0, 0), (1, 0), (0, 1), (1, 1)]]
        NW = len(WINS)
        wstate = {}
        aos_all = {}

        def win_scores(w, kt):
            j, hs, qh = WINS[w]
            lo, hi = hs * HD, (hs + 1) * HD
            q0 = qh * QH
            # the projections the scores read are forced at NORMAL priority
            # (they are dependencies either way; at prio 0 they'd also block
            # later scores in the static PE order); the scores themselves go
            # at high priority so they beat attn@V / V-proj / outproj backlog
            force(qk_gen[(j, 1, kt // 4)])
            if kt == 0:
                force(qk_gen[(j, 0, 2 * qh)])
                force(qk_gen[(j, 0, 2 * qh + 1)])
            with tc.high_priority():
                ps = psp.tile([128, 1024], F32, tag="ps")
                for qc in range(2):
                    nc.tensor.matmul(
                        ps[:, qc * 512:(qc + 1) * 512],
                        qkT[j][lo:hi, 1, kt * 128:(kt + 1) * 128],
                        qkT[j][lo:hi, 0, q0 + qc * 512:q0 + (qc + 1) * 512],
                        start=True, stop=True)
            # one k-chunk of lookahead (emitted after this k-tile's
            # scores so it cannot delay them): the forced unit needs
            # slack for its matmuls + qkT copy to clear the backlog
            force(qk_gen[(j, 1, min(kt // 4 + 1, 3))])
            return ps

        def win_attnv(w, kt):
            j, hs, qh = WINS[w]
            st = wstate[w]
            if kt == 0:
                st["pu"] = [pup.tile([128, 4, HD + 1], F32, tag="puA",
                                     name="puA"),
                            pup.tile([128, 4, HD + 1], F32, tag="puB",
                                     name="puB")]
            force(v_gen[(j, kt)])
            for qt in range(QT):
                # start/stop once per PSUM bank: start=True zeroes the whole
                # 2KB zero-region, so only the first of the 4 co-banked
                # accumulators may issue it
                nc.tensor.matmul(
                    st["pu"][qt // 4][:, qt % 4, :],
                    st["ats"][kt][:, qt * 128:(qt + 1) * 128],
                    v1k[(j, kt)][:, hs, :],
                    start=(kt == 0 and qt % 4 == 0),
                    stop=(kt == TT - 1 and qt % 4 == 3))

        def win_normalize(w):
            # r = 1/rowsum per q partition, ao = pu * r (bf16); high
            # priority so it jumps queued DVE copies — it releases the pu
            # banks the next window's attn@V needs. ao is pair-packed
            # [128q, 8qt, 128f]: hs0 writes cols 0:64, hs1 cols 64:128.
            j, hs, qh = WINS[w]
            puA, puB = wstate[w]["pu"]
            with tc.high_priority():
                r = rp.tile([128, QT, 1], F32, tag="r")
                nc.vector.reciprocal(r[:, 0:4, 0], puA[:, :, HD])
                nc.vector.reciprocal(r[:, 4:8, 0], puB[:, :, HD])
                if hs == 0:
                    ao = aop.tile([128, QT, 128], BF16, tag="ao")
                    aos_all[(j, qh)] = ao
                else:
                    ao = aos_all[(j, qh)]
                lo, hi = hs * HD, (hs + 1) * HD
                nc.vector.tensor_mul(ao[:, 0:4, lo:hi], puA[:, :, 0:HD],
                                     r[:, 0:4, :].to_broadcast([128, 4, HD]))
                nc.vector.tensor_mul(ao[:, 4:8, lo:hi], puB[:, :, 0:HD],
                                     r[:, 4:8, :].to_broadcast([128, 4, HD]))
            del wstate[w]
            post_window(j, hs, qh)

        def post_window(j, hs, qh):
            if hs == 1:
                # both heads of this (pair, q-half) done: transpose asap
                if j == 3 and qh == 1:
                    transpose_pe(j, qh, aos_all[(j, qh)])
                else:
                    fillers.append(transpose_gen(j, qh, aos_all[(j, qh)], 0))
                    fillers.append(transpose_gen(j, qh, aos_all[(j, qh)], 1))
            if hs == 1 and qh == 1:
                if j == 1:
                    for tt in range(16):
                        fillers.append(outproj_gen((0, 1), tt, 0, y_ap))
                        fillers.append(outproj_gen((0, 1), tt, 1, y_ap))
                if j == 2:
                    for tt in range(16):
                        fillers.append(outproj_gen((2,), tt, 0, y2_ap))
                        fillers.append(outproj_gen((2,), tt, 1, y2_ap))

        # ---- main schedule ----
        # pair 0: the x-chunk-2/3-dependent projection units go AFTER the
        # early V units in the filler FIFO -- the pump head would otherwise
        # stall on their DMAs (~15us) while ready V work sits behind them
        queue_qkproj(0, order=[(1, 0), (0, 0), (0, 1), (1, 1)])
        for j in range(NPAIR):
            aoT[j] = aotp.tile([128, S], BF16, tag="aoT", name=f"aoT{j}")
        for kt in range(8):
            g = vproj_gen(0, kt)
            v_gen[(0, kt)] = g
            fillers.append(g)
        for i, (fs, tck) in enumerate([(1, 2), (1, 3), (0, 2), (0, 3)]):
            g = qkproj_gen(0, fs, tck, i + 4)
            qk_gen[(0, fs, tck)] = g
            fillers.append(g)
        for kt in range(8, TT):
            g = vproj_gen(0, kt)
            v_gen[(0, kt)] = g
            fillers.append(g)

        NS = NW * TT
        next_av = 0
        pss = win_scores(0, 0)
        for s in range(NS):
            w, kt = s // TT, s % TT
            if kt == 0:
                wstate[w] = {"ats": [None] * TT}
                if WINS[w][1:] == (0, 0) and WINS[w][0] + 1 < NPAIR:
                    queue_qkproj(WINS[w][0] + 1)
                if WINS[w][1:] == (0, 1) and WINS[w][0] + 1 < NPAIR:
                    # pair j+1's V units: queued two windows before its
                    # attn@V starts consuming them
                    queue_vproj(WINS[w][0] + 1)
            ns = s + 1
            ps_next = win_scores(ns // TT, ns % TT) if ns < NS else None
            at = atp.tile([128, 1024], BF16, tag="attn")
            nc.scalar.activation(
                at[:, :], pss[:, :],
                func=mybir.ActivationFunctionType.Exp,
                scale=0.125)
            wstate[w]["ats"][kt] = at
            pss = ps_next
            pump(2)
            # attn@V emission lag: ramp up over window 0 (deferring the
            # V-projection debt out of the PE-oversubscribed first windows),
            # repay gradually through the ACT-paced middle, lag 0 in the
            # last window so the tail starts as soon as exps finish.
            if w == NW - 1:
                lag = 0
            elif s < 32:
                lag = min(s, LAG_MAX)
            else:
                lag = max(1, LAG_MAX - (s - 32) // 9)
            lag_tgt = s - lag
            while next_av <= lag_tgt:
                win_attnv(next_av // TT, next_av % TT)
                if next_av % TT == TT - 1:
                    win_normalize(next_av // TT)
                next_av += 1
        # drain leftovers, then the whole of y3 as the tail: it runs
        # post-stream, when ACT/DVE/Pool are all free for the copies
        drain()
        for i, tt in enumerate(range(16)):
            emit_outproj_tail(tt, i)

    nc.compile()
    return nc


_NC = None


def get_nc():
    global _NC
    if _NC is None:
        _NC = build_program()
    return _NC


def make_in_maps(x, Wqkv, Wout):
    import ml_dtypes
    bf16 = ml_dtypes.bfloat16
    x = np.asarray(x, dtype=np.float32)
    Wqkv = np.asarray(Wqkv, dtype=np.float32).astype(bf16)
    Wout = np.asarray(Wout, dtype=np.float32).astype(bf16)

    def pairify(w):  # [D, FH] -> [NPAIR, 128, DT, 128]
        return np.ascontiguousarray(
            w.reshape(DT, 128, NPAIR, 128).transpose(2, 1, 0, 3))

    in_maps = []
    for b in range(B):
        xbt = x[b].T.astype(bf16)  # [D, S]
        xt = np.ascontiguousarray(
            xbt.reshape(DT, 128, 4, 512).transpose(2, 1, 0, 3))
        for hh in range(2):
            c0 = hh * FH
            wv = Wqkv[:, 2 * D + c0:2 * D + c0 + FH]  # [D, FH]
            in_maps.append({
                "xt": xt,
                "wq": pairify(Wqkv[:, c0:c0 + FH]),
                "wk": pairify(Wqkv[:, D + c0:D + c0 + FH]),
                "wv": pairify(wv),
                "wout": np.ascontiguousarray(Wout[c0:c0 + FH, :]),
            })
    return in_maps


def assemble(results):
    y = np.empty((B, S, D), dtype=np.float32)
    for b in range(B):
        acc = np.zeros((S, D), dtype=np.float32)
        for half in (2 * b, 2 * b + 1):
            for name in ("y", "y2", "y3"):
                acc += results[half][name].astype(np.float32)
        y[b] = acc
    return y


def kernel(x, attn_mask, Wqkv, bqkv, Wout, bout):
    for name, t in (("attn_mask", attn_mask), ("bqkv", bqkv), ("bout", bout)):
        if np.any(np.asarray(t)):
            raise NotImplementedError(f"kernel assumes {name} == 0")
    nc = get_nc()
    res = run_bass_kernel_spmd(nc, make_in_maps(x, Wqkv, Wout),
                               core_ids=list(range(N_CORES)))
    return assemble(res.results)


if __name__ == "__main__":
    rng = np.random.default_rng(0)
    x = rng.standard_normal((B, S, D), dtype=np.float32)
    Wqkv = (rng.standard_normal((D, 3 * D), dtype=np.float32) / np.sqrt(D)).astype(np.float32)
    Wout = (rng.standard_normal((D, D), dtype=np.float32) / np.sqrt(D)).astype(np.float32)
    zeros = np.zeros
    y = kernel(x, zeros((S, S), np.float32), Wqkv, zeros(3 * D, np.float32),
               Wout, zeros(D, np.float32))
    print("y", y.shape, y.dtype, float(np.abs(y).mean()))



# revision 74
# speedup vs baseline: 1.0017x; 1.0017x over previous
"""Trainium2 Bass kernel for nn_MultiHeadAttention_9878424780806.

Problem (hardcoded): B=4, S=2048, D=1024, H=16 heads (head_dim 64), fp32.
  y = softmax((x@Wq)(x@Wk)^T / 8 + mask) @ (x@Wv) @ Wout   (+ zero biases)

Sharding: 8 cores = 4 batches x 2 head-halves (8 heads per core). Each core
computes a partial y for its batch from its 8 heads; the host sums the two
partials per batch (the out-projection is linear in heads). Host feeds x
pre-transposed per batch (xT [D, S]) and converts everything to bf16
(tolerance 2e-2 >> bf16 matmul error; halves DMA traffic and SBUF).

Per-core structure: 16 attention windows (8 heads x 2 q-halves of 1024),
fused into one 256-step software-pipelined stream. At step s:
  scores(s+1) [PE, bf16 -> fp32 PSUM] -> exp(s) [ScalarE, scale=1/8, bf16
  out] -> filler chunks -> attn@[V|1](s-lag): the probs tile is the
  STATIONARY operand, so out pu[q, 65] puts q on partitions and the softmax
  row-sum (from the appended ones column) at free col 64 -- normalization
  is then a per-partition scalar multiply (DVE reciprocal + broadcast mul),
  no cross-partition traffic.
The exp stream is the critical path (1038ns/step: 1 elem/lane/cycle at
1.2GHz + fixed access overhead; PSUM bank budget forbids wider tiles) and
runs gap-free: startup is DMA-latency-trimmed (pair-major host layouts
give 2KB descriptors; the serial DMA engine carries only what the first
projections need, split so each matmul's d-tiles can start as they land),
and V-projections are emitted per (pair, k-tile) so each pair's 16 units
are consumed by its own windows -- projecting all heads up front
concentrated a ~27us PE debt in window 0 and starved ScalarE. attn@V
emission lags the exp stream (ramp to ~20 steps, repaid through the
middle) to push that debt into per-step slack.

Normalized ao (pair-packed [128q, qt, 128f], both heads) is transposed by
PE into aoT [128f, t]. Out-projection runs in pair-groups writing three
outputs (y = pairs 0+1, y2 = pair 2, y3 = pair 3; host sums them): y/y2
stream out as fillers; ALL of y3 runs post-stream as the tail, when
ScalarE/DVE are free to alternate full-width PSUM->SBUF copies (GPSIMD
cannot touch PSUM on hw). Filler generators yield every ~2 matmuls and are
pumped 2 chunks/step in FIFO order arranged to avoid head-of-line blocking
on late x-chunk DMAs; scores matmuls are emitted at high priority so they
always beat the backlog. PSUM: scores 2x2 banks, pu 2x1, two 1-bank
scratch slots; the tail rotates over scores+scratch (3 slots).

All matmuls are bf16 (1 col/cycle at any width; rel-err ~5e-3 vs the 2e-2
gate); attn_mask and biases are all-zero by construction (spec fill=zeros);
kernel() refuses non-zero values.

Cost-model timeline: 310840ns (session start) -> 296444ns: exp stream is
gap-free from first exp at 12.8us to ~280us; y3 tail (ScalarE-leaning
full-width copies + serial 11.6us DMA drain) + close-out ~16.5us.
"""

import numpy as np
from collections import deque
from contextlib import ExitStack

import concourse.bass as bass
import concourse.tile as tile
from concourse import bacc, mybir
from concourse.bass_utils import run_bass_kernel_spmd
from concourse.masks import make_identity

F32 = mybir.dt.float32
BF16 = mybir.dt.bfloat16

B, S, D, H = 4, 2048, 1024, 16
HD = 64          # head dim
HPC = 8          # heads per core
N_CORES = 8

TT = S // 128    # 16 token tiles
DT = D // 128    # 8 d tiles
FH = HPC * HD    # 512 features per core half
NPAIR = HPC // 2
QH = S // 2      # 1024-wide q half
QT = QH // 128   # 8 q tiles per half
LAG_MAX = 20     # max attn@V emission lag (steps); bounded by atp bufs

# Host-side layouts are chosen so every load's innermost contiguous run is
# >= 2KB (descriptor elem >= 512B avoids the 2x DMA latency multiplier):
#   xt   [4, 128, DT, 512]  chunk-major x^T   (c,p,dt,t) = x.T[dt*128+p, c*512+t]
#   wq/wk[NPAIR, 128, DT, 128] pair-major     (j,p,dt,f) = W[dt*128+p, j*128+f]
#   wv   [128, DT, FH]                        (p,dt,f)   = Wv[dt*128+p, f]
#   wout [FH, D]
IN_SPECS = [("xt", [4, 128, DT, 512], BF16),
            ("wq", [NPAIR, 128, DT, 128], BF16),
            ("wk", [NPAIR, 128, DT, 128], BF16),
            ("wv", [NPAIR, 128, DT, 128], BF16),
            ("wout", [FH, D], BF16)]
OUT_SPECS = [("y", [S, D], BF16), ("y2", [S, D], BF16),
             ("y3", [S, D], BF16)]


def build_program():
    nc = bacc.Bacc("TRN2", target_bir_lowering=False, debug=False,
                   enable_asserts=False, num_devices=N_CORES)

    xt_ap = nc.dram_tensor("xt", [4, 128, DT, 512], BF16,
                           kind="ExternalInput").ap()
    wq_ap = nc.dram_tensor("wq", [NPAIR, 128, DT, 128], BF16,
                           kind="ExternalInput").ap()
    wk_ap = nc.dram_tensor("wk", [NPAIR, 128, DT, 128], BF16,
                           kind="ExternalInput").ap()
    wv_ap = nc.dram_tensor("wv", [NPAIR, 128, DT, 128], BF16,
                           kind="ExternalInput").ap()
    wout_ap = nc.dram_tensor("wout", [FH, D], BF16, kind="ExternalInput").ap()
    y_ap = nc.dram_tensor("y", [S, D], BF16, kind="ExternalOutput").ap()
    y2_ap = nc.dram_tensor("y2", [S, D], BF16, kind="ExternalOutput").ap()
    y3_ap = nc.dram_tensor("y3", [S, D], BF16, kind="ExternalOutput").ap()

    wout_r = wout_ap.rearrange("(ft p) e -> p ft e", p=128)

    with tile.TileContext(nc) as tc, ExitStack() as ctx:
        # SBUF pools
        xtp = ctx.enter_context(tc.tile_pool(name="xTp", bufs=1))
        wqkp = ctx.enter_context(tc.tile_pool(name="wqkp", bufs=2))
        wvp = ctx.enter_context(tc.tile_pool(name="wvp", bufs=1))
        wop = ctx.enter_context(tc.tile_pool(name="wop", bufs=1))
        qkp = ctx.enter_context(tc.tile_pool(name="qkp", bufs=2))
        v1p = ctx.enter_context(tc.tile_pool(name="v1p", bufs=1))
        atp = ctx.enter_context(tc.tile_pool(name="atp", bufs=34))
        aop = ctx.enter_context(tc.tile_pool(name="aop", bufs=6))
        aotp = ctx.enter_context(tc.tile_pool(name="aotp", bufs=3))
        rp = ctx.enter_context(tc.tile_pool(name="rp", bufs=4))
        ysbp = ctx.enter_context(tc.tile_pool(name="ysbp", bufs=8))
        idp = ctx.enter_context(tc.tile_pool(name="idp", bufs=1))
        # PSUM pools: 4 + 2 + 1 + 1 = 8 banks exactly
        psp = ctx.enter_context(tc.tile_pool(name="psp", bufs=2, space="PSUM"))
        pup = ctx.enter_context(tc.tile_pool(name="pup", bufs=1, space="PSUM"))
        psyp = ctx.enter_context(tc.tile_pool(name="psyp", bufs=1, space="PSUM"))
        psqp = ctx.enter_context(tc.tile_pool(name="psqp", bufs=1, space="PSUM"))

        # ramp the PE p-state while the input DMAs land: enough dummy
        # matmuls to still be running when the first projection starts,
        # so the ramp (3us of continuous busy) carries into real work.
        # A memset dummy (not the identity) so PE isn't gated on the
        # DVE-built identity tile.
        dummy = idp.tile([1, 128], BF16, tag="dummy")
        nc.vector.memset(dummy[:, :], 0.0)
        pwarm = psyp.tile([128, 128], F32, tag="psy", name="pwarm")
        for _ in range(46):
            nc.tensor.matmul(pwarm[:, :], dummy[0:1, :], dummy[0:1, :],
                             start=True, stop=True)

        # ---- loads ----
        # Emission order = DMA-engine order; the serial DMA engine is the
        # startup critical path: k0 and xt0 gate the first k-projection,
        # wq0 the q-projections, xt1 the second q chunk.
        wqk_t = [None] * NPAIR

        def emit_wqk_load(j):
            # [128, {q,k}, dt, 128]: each half is contiguous per partition
            # (2KB descriptors); k half first (k-proj is the first consumer)
            wqk = wqkp.tile([128, 2, DT, 128], BF16, tag="wqk",
                            name=f"wqk{j}")
            nc.sync.dma_start(wqk[:, 1], wk_ap[j])
            nc.sync.dma_start(wqk[:, 0], wq_ap[j])
            wqk_t[j] = wqk

        xTc = [xtp.tile([128, DT, 512], BF16, tag=f"xt{c}", name=f"xt{c}")
               for c in range(4)]
        wqk0 = wqkp.tile([128, 2, DT, 128], BF16, tag="wqk", name="wqk0")
        wqk_t[0] = wqk0
        nc.sync.dma_start(wqk0[:, 1], wk_ap[0])
        # x chunks 0/1 in quarter-DMAs: same serial transfer time, but the
        # first projections' tail (matmuls after the LAST piece lands)
        # shrinks from 4 d-tiles to 1, pulling the first exp ~0.5us earlier
        for q in range(4):
            nc.sync.dma_start(xTc[0][:, 2 * q:2 * q + 2, :],
                              xt_ap[0, :, 2 * q:2 * q + 2, :])
        nc.sync.dma_start(wqk0[:, 0], wq_ap[0])
        for q in range(4):
            nc.sync.dma_start(xTc[1][:, 2 * q:2 * q + 2, :],
                              xt_ap[1, :, 2 * q:2 * q + 2, :])

        # wv split per pair: pair 0's slice rides ahead of the cold x
        # chunks (its V-projections start first); the rest follows xt2/xt3
        # so the k-projections for late k-tiles aren't starved
        wv_t = wvp.tile([128, NPAIR, DT, 128], BF16, tag="wv")
        nc.sync.dma_start(wv_t[:, 0], wv_ap[0])

        for c in range(2, 4):
            nc.sync.dma_start(xTc[c][:, :, :], xt_ap[c])

        for j in range(1, NPAIR):
            nc.sync.dma_start(wv_t[:, j], wv_ap[j])

        wout_t = []
        for j in range(NPAIR):
            w = wop.tile([128, 1024], BF16, tag=f"wout{j}")
            nc.sync.dma_start(w[:, :], wout_r[:, j, :])
            wout_t.append(w)

        idt = idp.tile([128, 128], BF16, tag="id")
        make_identity(nc, idt[:, :])

        # preload the Exp activation table while DMAs are in flight
        warm = rp.tile([1, 2], F32, tag="warm")
        nc.vector.memset(warm[:, :], 0.0)
        nc.scalar.activation(warm[:, 1:2], warm[:, 0:1],
                             func=mybir.ActivationFunctionType.Exp)

        # ---- filler machinery ----
        # Fillers are generators that yield every ~2 matmuls (~430ns of PE
        # work). The attention windows pump one chunk per k-tile, matching
        # the PE slack left over while ScalarE exps; hard dependencies
        # (projections an upcoming scores matmul needs) are force-completed.
        fillers = deque()
        _cur = [None]

        def pump(chunks=1):
            while chunks > 0:
                if _cur[0] is None:
                    if not fillers:
                        return
                    _cur[0] = fillers.popleft()
                try:
                    next(_cur[0])
                    chunks -= 1
                except StopIteration:
                    _cur[0] = None

        def force(gen):
            for _ in gen:
                pass

        def drain():
            pump(1 << 30)

        # QK projection: qkT[j] [128f, {q,k}, S]
        qkT = [None] * NPAIR
        qk_gen = {}

        def qkproj_gen(j, fs, tck, sel):
            pool, tg = (psqp, "psq") if sel % 2 == 0 else (psyp, "psy")
            psq = pool.tile([128, 512], F32, tag=tg, name="psq")
            for dt in range(DT):
                nc.tensor.matmul(
                    psq[:, :],
                    wqk_t[j][:, fs, dt, :],
                    xTc[tck][:, dt, :],
                    start=(dt == 0), stop=(dt == DT - 1))
                if dt % 2 == 1 and dt < DT - 1:
                    yield
            nc.vector.tensor_copy(
                qkT[j][:, fs, tck * 512:(tck + 1) * 512], psq[:, :])
            yield

        def queue_qkproj(j, order=None):
            if j > 0:
                emit_wqk_load(j)
            qkT[j] = qkp.tile([128, 2, S], BF16, tag="qkT", name=f"qkT{j}")
            # k (fs=1) before q, early token chunks first: matches the order
            # the first window consumes them in
            for i, (fs, tck) in enumerate(order or
                                          [(1, 0), (0, 0), (0, 1), (1, 1),
                                           (1, 2), (1, 3), (0, 2), (0, 3)]):
                g = qkproj_gen(j, fs, tck, i)
                qk_gen[(j, fs, tck)] = g
                fillers.append(g)

        # V projection per (pair, k-tile): v1k[(j,kt)] [128t, 2h, 65] =
        # [V | 1]. Split per pair so pair j's 16 units are only consumed by
        # its own 4 windows -- projecting all 8 heads up front concentrated
        # a ~27us PE debt into window 0 and stalled the exp stream early.
        v1k = {}
        v_gen = {}

        def vproj_gen(j, kt):
            v1 = v1p.tile([128, 2, HD + 1], BF16, tag=f"v1_{j}_{kt}",
                          name=f"v1_{j}_{kt}")
            v1k[(j, kt)] = v1
            nc.vector.memset(v1[:, :, HD], 1.0)
            xc, sub = xTc[kt // 4], kt % 4
            pool, tg = (psqp, "psq") if kt % 2 == 0 else (psyp, "psy")
            psv = pool.tile([128, 128], F32, tag=tg, name="psv")
            for dt in range(DT):
                nc.tensor.matmul(psv[:, :],
                                 xc[:, dt, sub * 128:(sub + 1) * 128],
                                 wv_t[:, j, dt, :],
                                 start=(dt == 0), stop=(dt == DT - 1))
                if dt % 2 == 1 and dt < DT - 1:
                    yield
            nc.vector.tensor_copy(
                v1[:, :, 0:HD],
                psv[:, :].rearrange("p (h c) -> p h c", c=HD))
            yield

        def queue_vproj(j):
            for kt in range(TT):
                g = vproj_gen(j, kt)
                v_gen[(j, kt)] = g
                fillers.append(g)

        # Transposes: pair-packed ao [128q, 8qt, 128f] -> aoT[pair] [128f, S]
        # via PE transpose ([128,128] per qt) into scratch PSUM, then one DVE
        # copy per 4-qt half.
        aoT = [None] * NPAIR

        def transpose_gen(j, qh, ao, half):
            pool, tg = (psqp, "psq") if half == 0 else (psyp, "psy")
            trT = pool.tile([128, 4, 128], BF16, tag=tg, name="trT")
            for qi in range(4):
                qt = half * 4 + qi
                nc.tensor.transpose(trT[:, qi, :], ao[:, qt, :], idt[:, :])
                if qi == 1:
                    yield
            nc.vector.tensor_copy(
                aoT[j].rearrange("p (t c) -> p t c", c=128)
                    [:, qh * 8 + half * 4:qh * 8 + half * 4 + 4, :],
                trT[:, :, :])
            yield

        def transpose_pe(j, qh, ao):
            # synchronous variant for the tail unit
            for half in range(2):
                for _ in transpose_gen(j, qh, ao, half):
                    pass

        # Out-projection in pair-groups so most units can run as filler while
        # attention is still in flight: pairs {0,1} write y, then {2} and {3}
        # DMA-accumulate on top as their aoT becomes available.
        def outproj_gen(pairs, tt, ec, target):
            pool, tg = (psyp, "psy") if (tt + ec) % 2 == 0 else (psqp, "psq")
            psy = pool.tile([128, 512], F32, tag=tg, name="psy")
            for i, j in enumerate(pairs):
                nc.tensor.matmul(
                    psy[:, :],
                    aoT[j].rearrange("p (t c) -> p t c", c=128)[:, tt, :],
                    wout_t[j][:, ec * 512:(ec + 1) * 512],
                    start=(i == 0), stop=(i == len(pairs) - 1))
            ysb = ysbp.tile([128, 512], BF16, tag="y", name="ysb")
            nc.vector.tensor_copy(ysb[:, :], psy[:, :])
            nc.sync.dma_start(
                target[tt * 128:(tt + 1) * 128, ec * 512:(ec + 1) * 512],
                ysb[:, :])
            yield

        # Tail variant (pair 3, q-half 1): attention is done, so
        # double-buffer across the (now free) scores pool. Copies are split
        # per 512-col half and spread over Pool+DVE+ACT so the copy latency
        # (the tail's pacing term) is ~650ns instead of 1.2us, and each half
        # DMAs out as soon as its copy lands.
        def tail_copy(eng, dst, src):
            # GPSIMD cannot read PSUM on hw: ACT/DVE only
            [nc.scalar.copy, nc.vector.tensor_copy][eng % 2](dst, src)

        def emit_outproj_tail(tt, i):
            aoTv = aoT[3].rearrange("p (t c) -> p t c", c=128)
            if i % 3 == 2:
                halves = [psqp.tile([128, 512], F32, tag="psq", name="pst"),
                          psyp.tile([128, 512], F32, tag="psy", name="pst")]
                wide = None
            else:
                psy = psp.tile([128, 1024], F32, tag="ps", name="psy2")
                halves = [psy[:, 0:512], psy[:, 512:1024]]
                wide = psy
            ysb = ysbp.tile([128, 1024], BF16, tag="ytail", name="ysb2")
            for ec in range(2):
                nc.tensor.matmul(
                    halves[ec], aoTv[:, tt, :],
                    wout_t[3][:, ec * 512:(ec + 1) * 512],
                    start=True, stop=True)
            if wide is not None:
                # one full-width copy amortizes the per-instruction access
                # overhead. The first three tiles go to ScalarE: it frees the
                # moment the last exp retires, while DVE still carries the
                # final normalize + transpose copies; later tiles lean 2-of-3
                # onto ScalarE (its copy is 150ns cheaper than DVE's).
                tail_copy(0 if i < 3 or i % 3 != 1 else 1, ysb[:, :], wide[:, :])
            else:
                tail_copy(i, ysb[:, 0:512], halves[0])
                tail_copy(i + 1, ysb[:, 512:1024], halves[1])
            nc.sync.dma_start(y3_ap[tt * 128:(tt + 1) * 128, :], ysb[:, :])

        # ---- attention stream: 16 windows (pair-major, both-heads-q0
        # first) fused into one 256-step software pipeline. At step s:
        # scores(s+1) -> exp(s) -> filler chunk -> attn@V(s-1); window
        # boundaries are pipelined over just like k-tile boundaries.
        WINS = [(j, hs, qh) for j in range(NPAIR)
                for hs, qh in [(0, 0), (1, 0), (0, 1), (1, 1)]]
        NW = len(WINS)
        wstate = {}
        aos_all = {}

        def win_scores(w, kt):
            j, hs, qh = WINS[w]
            lo, hi = hs * HD, (hs + 1) * HD
            q0 = qh * QH
            # the projections the scores read are forced at NORMAL priority
            # (they are dependencies either way; at prio 0 they'd also block
            # later scores in the static PE order); the scores themselves go
            # at high priority so they beat attn@V / V-proj / outproj backlog
            force(qk_gen[(j, 1, kt // 4)])
            if kt == 0:
                force(qk_gen[(j, 0, 2 * qh)])
                force(qk_gen[(j, 0, 2 * qh + 1)])
            with tc.high_priority():
                ps = psp.tile([128, 1024], F32, tag="ps")
                for qc in range(2):
                    nc.tensor.matmul(
                        ps[:, qc * 512:(qc + 1) * 512],
                        qkT[j][lo:hi, 1, kt * 128:(kt + 1) * 128],
                        qkT[j][lo:hi, 0, q0 + qc * 512:q0 + (qc + 1) * 512],
                        start=True, stop=True)
            # one k-chunk of lookahead (emitted after this k-tile's
            # scores so it cannot delay them): the forced unit needs
            # slack for its matmuls + qkT copy to clear the backlog
            force(qk_gen[(j, 1, min(kt // 4 + 1, 3))])
            return ps

        def win_attnv(w, kt):
            j, hs, qh = WINS[w]
            st = wstate[w]
            if kt == 0:
                st["pu"] = [pup.tile([128, 4, HD + 1], F32, tag="puA",
                                     name="puA"),
                            pup.tile([128, 4, HD + 1], F32, tag="puB",
                                     name="puB")]
            force(v_gen[(j, kt)])
            for qt in range(QT):
                # start/stop once per PSUM bank: start=True zeroes the whole
                # 2KB zero-region, so only the first of the 4 co-banked
                # accumulators may issue it
                nc.tensor.matmul(
                    st["pu"][qt // 4][:, qt % 4, :],
                    st["ats"][kt][:, qt * 128:(qt + 1) * 128],
                    v1k[(j, kt)][:, hs, :],
                    start=(kt == 0 and qt % 4 == 0),
                    stop=(kt == TT - 1 and qt % 4 == 3))

        def win_normalize(w):
            # r = 1/rowsum per q partition, ao = pu * r (bf16); high
            # priority so it jumps queued DVE copies — it releases the pu
            # banks the next window's attn@V needs. ao is pair-packed
            # [128q, 8qt, 128f]: hs0 writes cols 0:64, hs1 cols 64:128.
            j, hs, qh = WINS[w]
            puA, puB = wstate[w]["pu"]
            with tc.high_priority():
                r = rp.tile([128, QT, 1], F32, tag="r")
                nc.vector.reciprocal(r[:, 0:4, 0], puA[:, :, HD])
                nc.vector.reciprocal(r[:, 4:8, 0], puB[:, :, HD])
                if hs == 0:
                    ao = aop.tile([128, QT, 128], BF16, tag="ao")
                    aos_all[(j, qh)] = ao
                else:
                    ao = aos_all[(j, qh)]
                lo, hi = hs * HD, (hs + 1) * HD
                nc.vector.tensor_mul(ao[:, 0:4, lo:hi], puA[:, :, 0:HD],
                                     r[:, 0:4, :].to_broadcast([128, 4, HD]))
                nc.vector.tensor_mul(ao[:, 4:8, lo:hi], puB[:, :, 0:HD],
                                     r[:, 4:8, :].to_broadcast([128, 4, HD]))
            del wstate[w]
            post_window(j, hs, qh)

        def post_window(j, hs, qh):
            if hs == 1:
                # both heads of this (pair, q-half) done: transpose asap
                if j == 3 and qh == 1:
                    transpose_pe(j, qh, aos_all[(j, qh)])
                else:
                    fillers.append(transpose_gen(j, qh, aos_all[(j, qh)], 0))
                    fillers.append(transpose_gen(j, qh, aos_all[(j, qh)], 1))
            if hs == 1 and qh == 1:
                if j == 1:
                    for tt in range(16):
                        fillers.append(outproj_gen((0, 1), tt, 0, y_ap))
                        fillers.append(outproj_gen((0, 1), tt, 1, y_ap))
                if j == 2:
                    for tt in range(16):
                        fillers.append(outproj_gen((2,), tt, 0, y2_ap))
                        fillers.append(outproj_gen((2,), tt, 1, y2_ap))

        # ---- main schedule ----
        # pair 0: the x-chunk-2/3-dependent projection units go AFTER the
        # early V units in the filler FIFO -- the pump head would otherwise
        # stall on their DMAs (~15us) while ready V work sits behind them
        queue_qkproj(0, order=[(1, 0), (0, 0), (0, 1), (1, 1)])
        for j in range(NPAIR):
            aoT[j] = aotp.tile([128, S], BF16, tag="aoT", name=f"aoT{j}")
        for kt in range(8):
            g = vproj_gen(0, kt)
            v_gen[(0, kt)] = g
            fillers.append(g)
        for i, (fs, tck) in enumerate([(1, 2), (1, 3), (0, 2), (0, 3)]):
            g = qkproj_gen(0, fs, tck, i + 4)
            qk_gen[(0, fs, tck)] = g
            fillers.append(g)
        for kt in range(8, TT):
            g = vproj_gen(0, kt)
            v_gen[(0, kt)] = g
            fillers.append(g)

        NS = NW * TT
        next_av = 0
        pss = win_scores(0, 0)
        for s in range(NS):
            w, kt = s // TT, s % TT
            if kt == 0:
                wstate[w] = {"ats": [None] * TT}
                if WINS[w][1:] == (0, 0) and WINS[w][0] + 1 < NPAIR:
                    queue_qkproj(WINS[w][0] + 1)
                if WINS[w][1:] == (0, 1) and WINS[w][0] + 1 < NPAIR:
                    # pair j+1's V units: queued two windows before its
                    # attn@V starts consuming them
                    queue_vproj(WINS[w][0] + 1)
            ns = s + 1
            ps_next = win_scores(ns // TT, ns % TT) if ns < NS else None
            at = atp.tile([128, 1024], BF16, tag="attn")
            nc.scalar.activation(
                at[:, :], pss[:, :],
                func=mybir.ActivationFunctionType.Exp,
                scale=0.125)
            wstate[w]["ats"][kt] = at
            pss = ps_next
            pump(2)
            # attn@V emission lag: ramp up over window 0 (deferring the
            # V-projection debt out of the PE-oversubscribed first windows),
            # repay gradually through the ACT-paced middle, lag 0 in the
            # last window so the tail starts as soon as exps finish.
            if w == NW - 1:
                lag = 0
            elif s < 32:
                lag = min(s, LAG_MAX)
            else:
                lag = max(1, LAG_MAX - (s - 32) // 9)
            lag_tgt = s - lag
            while next_av <= lag_tgt:
                win_attnv(next_av // TT, next_av % TT)
                if next_av % TT == TT - 1:
                    win_normalize(next_av // TT)
                next_av += 1
        # drain leftovers, then the whole of y3 as the tail: it runs
        # post-stream, when ACT/DVE/Pool are all free for the copies
        drain()
        for i, tt in enumerate(range(16)):
            emit_outproj_tail(tt, i)

    nc.compile()
    return nc


_NC = None


def get_nc():
    global _NC
    if _NC is None:
        _NC = build_program()
    return _NC


def make_in_maps(x, Wqkv, Wout):
    import ml_dtypes
    bf16 = ml_dtypes.bfloat16
    x = np.asarray(x, dtype=np.float32)
    Wqkv = np.asarray(Wqkv, dtype=np.float32).astype(bf16)
    Wout = np.asarray(Wout, dtype=np.float32).astype(bf16)

    def pairify(w):  # [D, FH] -> [NPAIR, 128, DT, 128]
        return np.ascontiguousarray(
            w.reshape(DT, 128, NPAIR, 128).transpose(2, 1, 0, 3))

    in_maps = []
    for b in range(B):
        xbt = x[b].T.astype(bf16)  # [D, S]
        xt = np.ascontiguousarray(
            xbt.reshape(DT, 128, 4, 512).transpose(2, 1, 0, 3))
        for hh in range(2):
            c0 = hh * FH
            wv = Wqkv[:, 2 * D + c0:2 * D + c0 + FH]  # [D, FH]
            in_maps.append({
                "xt": xt,
                "wq": pairify(Wqkv[:, c0:c0 + FH]),
                "wk": pairify(Wqkv[:, D + c0:D + c0 + FH]),
                "wv": pairify(wv),
                "wout": np.ascontiguousarray(Wout[c0:c0 + FH, :]),
            })
    return in_maps


def assemble(results):
    y = np.empty((B, S, D), dtype=np.float32)
    for b in range(B):
        acc = np.zeros((S, D), dtype=np.float32)
        for half in (2 * b, 2 * b + 1):
            for name in ("y", "y2", "y3"):
                acc += results[half][name].astype(np.float32)
        y[b] = acc
    return y


def kernel(x, attn_mask, Wqkv, bqkv, Wout, bout):
    for name, t in (("attn_mask", attn_mask), ("bqkv", bqkv), ("bout", bout)):
        if np.any(np.asarray(t)):
            raise NotImplementedError(f"kernel assumes {name} == 0")
    nc = get_nc()
    res = run_bass_kernel_spmd(nc, make_in_maps(x, Wqkv, Wout),
                               core_ids=list(range(N_CORES)))
    return assemble(res.results)


if __name__ == "__main__":
    rng = np.random.default_rng(0)
    x = rng.standard_normal((B, S, D), dtype=np.float32)
    Wqkv = (rng.standard_normal((D, 3 * D), dtype=np.float32) / np.sqrt(D)).astype(np.float32)
    Wout = (rng.standard_normal((D, D), dtype=np.float32) / np.sqrt(D)).astype(np.float32)
    zeros = np.zeros
    y = kernel(x, zeros((S, S), np.float32), Wqkv, zeros(3 * D, np.float32),
               Wout, zeros(D, np.float32))
    print("y", y.shape, y.dtype, float(np.abs(y).mean()))

